# revision 4
# baseline (speedup 1.0000x reference)
"""Trainium2 Bass kernel for nn_NeuralFODE.

Math: the reference MLP has no activations between its four linear layers,
so the whole MLP collapses to one affine map:

    deriv_i = tanh([t_i, y_i] @ Weff + beff),   Weff = W0@W1@W2@W3  (65x64)
    y_{i+1} = y_i + c_i * deriv_i,              c_i = sqrt(dt_i)/Gamma(0.5)

Split Weff into the t-row (w_t, 64) and the y-block (Wy, 64x64) and define
g_i = t_i*w_t + beff; then each step is

    u_i = tanh(z_i + g_i),  z_i = y_i @ Wy,  y_{i+1} = y_i + c_i*u_i.

Device layout (feature dim on partitions, batch on the free dim):
  - z is kept transposed in a PSUM accumulator Z [64f x 64b]; each step the
    tensor engine accumulates  Z += (cbar*Wy)^T-stationary @ u_i, which equals
    z_{i+1} = z_i + (c_i u_i) @ Wy up to the (negligible) c_i != cbar jitter.
  - ScalarE computes u_i = tanh(Z + g[:, i]) straight out of PSUM with the
    per-partition bias AP, writing u_i to SBUF.
  - VectorE maintains the exact fp32 y track (y += c_i*u_i, per-partition
    scalar c from SBUF) off the critical path; every REFRESH steps the PE
    re-derives Z from the exact y (start=True) to bound accumulation drift.
  - Each y_{i+1} is DMA'd to DRAM as traj[i] = y^T [64f x 64b].

Sharding: data-parallel over batch: 8 cores x 64 batch rows, weights
replicated, SPMD (same NEFF, per-core xT slice). Host transposes the
gathered per-core [1023, 64f, 64b] outputs into the [512, 1024, 64] result.
"""

import math

import numpy as np

B, T, D = 512, 1024, 64
NCORES = 8
BC = B // NCORES          # batch rows per core
NSTEP = T - 1             # 1023 recurrence steps
REFRESH = 64              # re-derive Z from the exact y track every N steps
GAMMA_ALPHA = math.gamma(0.5)

_CACHE = {}


def _build_nc():
    """Build + compile the (input-independent) Bass program once."""
    import concourse.bacc as bacc
    import concourse.bass as bass
    import concourse.tile as tile
    from concourse import mybir

    dt = mybir.dt.float32
    nc = bacc.Bacc(
        "TRN2", target_bir_lowering=False, debug=False, num_devices=NCORES
    )

    xT_d = nc.dram_tensor("xT", [D, BC], dt, kind="ExternalInput")
    wy_d = nc.dram_tensor("Wy", [D, D], dt, kind="ExternalInput")
    cwy_d = nc.dram_tensor("cWy", [D, D], dt, kind="ExternalInput")
    g_d = nc.dram_tensor("g", [D, NSTEP], dt, kind="ExternalInput")
    c_d = nc.dram_tensor("crep", [D, NSTEP], dt, kind="ExternalInput")
    traj_d = nc.dram_tensor("traj", [NSTEP, D, BC], dt, kind="ExternalOutput")

    # mm plan: mm[0] is the prologue (rhs=y0); mm[i] (i>=1) runs after ACT_{i-1}.
    # Refresh mms restart the PSUM accumulation group from the exact y track.
    is_refresh = [False] * NSTEP
    for k in range(1, NSTEP):
        is_refresh[k] = (k % REFRESH) == 0
    # stop=True on the last mm of each accumulation group (sim bookkeeping).
    is_stop = [
        k == NSTEP - 1 or is_refresh[k + 1] for k in range(NSTEP)
    ]

    with tile.TileContext(nc) as tc:
        with (
            tc.tile_pool(name="const", bufs=1) as const,
            tc.tile_pool(name="ybuf", bufs=2) as ybuf,
            tc.tile_pool(name="ubuf", bufs=3) as ubuf,
            tc.tile_pool(name="vbuf", bufs=3) as vbuf,
            tc.tile_pool(name="psum", bufs=1, space=bass.MemorySpace.PSUM) as psum,
        ):
            wy = const.tile([D, D], dt)
            nc.sync.dma_start(wy[:], wy_d[:])
            cwy = const.tile([D, D], dt)
            nc.sync.dma_start(cwy[:], cwy_d[:])
            g = const.tile([D, NSTEP], dt)
            nc.sync.dma_start(g[:], g_d[:])
            cr = const.tile([D, NSTEP], dt)
            nc.sync.dma_start(cr[:], c_d[:])
            y = ybuf.tile([D, BC], dt, tag="y")
            nc.sync.dma_start(y[:], xT_d[:])
            Z = psum.tile([D, BC], dt)

            nc.tensor.matmul(Z[:], wy[:], y[:], start=True, stop=is_stop[0])
            for i in range(NSTEP):
                u = ubuf.tile([D, BC], dt, tag="u")
                nc.scalar.activation(
                    u[:],
                    Z[:],
                    mybir.ActivationFunctionType.Tanh,
                    bias=g[:, i : i + 1],
                )
                # exact y track (VectorE, off the ACT<->PE critical path)
                v = vbuf.tile([D, BC], dt, tag="v")
                nc.vector.tensor_scalar_mul(v[:], u[:], cr[:, i : i + 1])
                ynew = ybuf.tile([D, BC], dt, tag="y")
                nc.vector.tensor_add(ynew[:], y[:], v[:])
                nc.sync.dma_start(traj_d[i], ynew[:])
                y = ynew
                if i + 1 < NSTEP:
                    k = i + 1
                    if is_refresh[k]:
                        nc.tensor.matmul(
                            Z[:], wy[:], y[:], start=True, stop=is_stop[k]
                        )
                    else:
                        nc.tensor.matmul(
                            Z[:], cwy[:], u[:], start=False, stop=is_stop[k]
                        )

    nc.compile()
    return nc


def _host_prep(x, t, W0, b0, W1, b1, W2, b2, W3, b3):
    """Collapse the linear MLP in float64 and build per-core device inputs."""
    f8 = np.float64
    W0d, W1d, W2d, W3d = (w.astype(f8) for w in (W0, W1, W2, W3))
    b0d, b1d, b2d, b3d = (b.astype(f8) for b in (b0, b1, b2, b3))
    Weff = W0d @ W1d @ W2d @ W3d                      # [65, 64]
    beff = ((b0d @ W1d + b1d) @ W2d + b2d) @ W3d + b3d
    w_t = Weff[0]                                      # [64]
    Wyd = Weff[1:]                                     # [64, 64]

    t32 = t.astype(np.float32)
    dt32 = (t32[1:] - t32[:-1]).astype(np.float32)
    c32 = (np.sqrt(dt32) / np.float32(GAMMA_ALPHA)).astype(np.float32)[:NSTEP]
    cbar = f8(np.median(c32))

    Wy32 = np.ascontiguousarray(Wyd.astype(np.float32))
    cWy32 = np.ascontiguousarray((cbar * Wyd).astype(np.float32))
    g32 = np.ascontiguousarray(
        (t.astype(f8)[:-1][None, :NSTEP] * w_t[:, None] + beff[:, None]).astype(
            np.float32
        )
    )                                                  # [64, 1023]
    crep = np.ascontiguousarray(
        np.broadcast_to(c32[None, :], (D, NSTEP)).astype(np.float32)
    )                                                  # [64, 1023]

    in_maps = []
    for c in range(NCORES):
        xc = np.ascontiguousarray(x[c * BC : (c + 1) * BC, :].T.astype(np.float32))
        in_maps.append(
            {"xT": xc, "Wy": Wy32, "cWy": cWy32, "g": g32, "crep": crep}
        )
    return in_maps, c32


def kernel(x, t, W0, b0, W1, b1, W2, b2, W3, b3):
    from concourse.bass_utils import run_bass_kernel_spmd

    if "nc" not in _CACHE:
        _CACHE["nc"] = _build_nc()
    nc = _CACHE["nc"]

    in_maps, _ = _host_prep(x, t, W0, b0, W1, b1, W2, b2, W3, b3)
    res = run_bass_kernel_spmd(nc, in_maps, core_ids=list(range(NCORES)))
    _CACHE["last_result"] = res

    sol = np.empty((B, T, D), np.float32)
    sol[:, 0, :] = x.astype(np.float32)
    for c in range(NCORES):
        # traj [NSTEP, 64f, 64b] -> [64b, NSTEP, 64f]
        sol[c * BC : (c + 1) * BC, 1:, :] = res.results[c]["traj"].transpose(
            2, 0, 1
        )
    return sol


# revision 9
# speedup vs baseline: 1.1456x; 1.1456x over previous
"""Trainium2 Bass kernel for nn_NeuralFODE.

Math: the reference MLP has no activations between its four linear layers,
so the whole MLP collapses to one affine map:

    deriv_i = tanh([t_i, y_i] @ Weff + beff),   Weff = W0@W1@W2@W3  (65x64)
    y_{i+1} = y_i + c_i * deriv_i,              c_i = sqrt(dt_i)/Gamma(0.5)

Split Weff into the t-row (w_t, 64) and the y-block (Wy, 64x64) and define
g_i = t_i*w_t + beff; then each step is

    u_i = tanh(z_i + g_i),  z_i = y_i @ Wy,  y_{i+1} = y_i + c_i*u_i.

Device layout (feature dim on partitions, batch on the free dim):
  - z is kept transposed in a PSUM accumulator Z [64f x 64b]; each step the
    tensor engine accumulates  Z += (cbar*Wy)^T-stationary @ u_i, which equals
    z_{i+1} = z_i + (c_i u_i) @ Wy up to the (negligible) c_i != cbar jitter.
  - ScalarE computes u_i = tanh(Z + g[:, i]) straight out of PSUM with the
    per-partition bias AP, writing u_i to SBUF.
  - VectorE maintains the exact fp32 y track (y += c_i*u_i, per-partition
    scalar c from SBUF) off the critical path; every REFRESH steps the PE
    re-derives Z from the exact y (start=True) to bound accumulation drift.
  - Each y_{i+1} is DMA'd to DRAM as traj[i] = y^T [64f x 64b].

Sharding: data-parallel over batch: 8 cores x 64 batch rows, weights
replicated, SPMD (same NEFF, per-core xT slice). Host transposes the
gathered per-core [1023, 64f, 64b] outputs into the [512, 1024, 64] result.
"""

import math
import os

import numpy as np

B, T, D = 512, 1024, 64
NCORES = 8
BC = B // NCORES          # batch rows per core
NSTEP = T - 1             # 1023 recurrence steps
REFRESH = 64              # re-derive Z from the exact y track every N steps
GAMMA_ALPHA = math.gamma(0.5)

_CACHE = {}


def _build_nc():
    """Build + compile the (input-independent) Bass program once."""
    import concourse.bacc as bacc
    import concourse.bass as bass
    import concourse.tile as tile
    from concourse import mybir

    dt = mybir.dt.float32
    # Experimental: single-pass reduced-precision fp32 matmul (TF32-like)
    # instead of the hardware's 2-pass full fp32. Off by default.
    f32r = os.environ.get("MM_F32R", "0") == "1"
    mmdt = mybir.dt.float32r if f32r else dt

    def mm_op(ap):
        return ap.bitcast(mmdt) if f32r else ap

    nc = bacc.Bacc(
        "TRN2", target_bir_lowering=False, debug=False, num_devices=NCORES
    )

    xT_d = nc.dram_tensor("xT", [D, BC], dt, kind="ExternalInput")
    wy_d = nc.dram_tensor("Wy", [D, D], dt, kind="ExternalInput")
    cwy_d = nc.dram_tensor("cWy", [D, D], dt, kind="ExternalInput")
    g_d = nc.dram_tensor("g", [D, NSTEP], dt, kind="ExternalInput")
    c_d = nc.dram_tensor("crep", [D, NSTEP], dt, kind="ExternalInput")
    traj_d = nc.dram_tensor("traj", [NSTEP, D, BC], dt, kind="ExternalOutput")

    # mm plan: mm[0] is the prologue (rhs=y0); mm[i] (i>=1) runs after ACT_{i-1}.
    # Refresh mms restart the PSUM accumulation group from the exact y track.
    is_refresh = [False] * NSTEP
    for k in range(1, NSTEP):
        is_refresh[k] = (k % REFRESH) == 0
    # stop=True on the last mm of each accumulation group (sim bookkeeping).
    is_stop = [
        k == NSTEP - 1 or is_refresh[k + 1] for k in range(NSTEP)
    ]

    with tile.TileContext(nc) as tc:
        with (
            tc.tile_pool(name="const", bufs=1) as const,
            tc.tile_pool(name="ybuf", bufs=4) as ybuf,
            tc.tile_pool(name="ubuf", bufs=8) as ubuf,
            tc.tile_pool(name="vbuf", bufs=8) as vbuf,
            tc.tile_pool(name="psum", bufs=1, space=bass.MemorySpace.PSUM) as psum,
        ):
            wy = const.tile([D, D], dt)
            nc.sync.dma_start(wy[:], wy_d[:])
            cwy = const.tile([D, D], dt)
            nc.sync.dma_start(cwy[:], cwy_d[:])
            g = const.tile([D, NSTEP], dt)
            nc.sync.dma_start(g[:], g_d[:])
            cr = const.tile([D, NSTEP], dt)
            nc.sync.dma_start(cr[:], c_d[:])
            y = ybuf.tile([D, BC], dt, tag="y")
            nc.sync.dma_start(y[:], xT_d[:])
            Z = psum.tile([D, BC], dt)

            nc.tensor.matmul(
                Z[:], mm_op(wy[:]), mm_op(y[:]), start=True, stop=is_stop[0]
            )
            for i in range(NSTEP):
                u = ubuf.tile([D, BC], dt, tag="u")
                nc.scalar.activation(
                    u[:],
                    Z[:],
                    mybir.ActivationFunctionType.Tanh,
                    bias=g[:, i : i + 1],
                )
                # exact y track (VectorE, off the ACT<->PE critical path)
                v = vbuf.tile([D, BC], dt, tag="v")
                nc.vector.tensor_scalar_mul(v[:], u[:], cr[:, i : i + 1])
                ynew = ybuf.tile([D, BC], dt, tag="y")
                nc.vector.tensor_add(ynew[:], y[:], v[:])
                nc.sync.dma_start(traj_d[i], ynew[:])
                y = ynew
                if i + 1 < NSTEP:
                    k = i + 1
                    if is_refresh[k]:
                        nc.tensor.matmul(
                            Z[:],
                            mm_op(wy[:]),
                            mm_op(y[:]),
                            start=True,
                            stop=is_stop[k],
                        )
                    else:
                        nc.tensor.matmul(
                            Z[:],
                            mm_op(cwy[:]),
                            mm_op(u[:]),
                            start=False,
                            stop=is_stop[k],
                        )

    nc.compile()
    return nc


def _host_prep(x, t, W0, b0, W1, b1, W2, b2, W3, b3):
    """Collapse the linear MLP in float64 and build per-core device inputs."""
    f8 = np.float64
    W0d, W1d, W2d, W3d = (w.astype(f8) for w in (W0, W1, W2, W3))
    b0d, b1d, b2d, b3d = (b.astype(f8) for b in (b0, b1, b2, b3))
    Weff = W0d @ W1d @ W2d @ W3d                      # [65, 64]
    beff = ((b0d @ W1d + b1d) @ W2d + b2d) @ W3d + b3d
    w_t = Weff[0]                                      # [64]
    Wyd = Weff[1:]                                     # [64, 64]

    t32 = t.astype(np.float32)
    dt32 = (t32[1:] - t32[:-1]).astype(np.float32)
    c32 = (np.sqrt(dt32) / np.float32(GAMMA_ALPHA)).astype(np.float32)[:NSTEP]
    cbar = f8(np.median(c32))

    Wy32 = np.ascontiguousarray(Wyd.astype(np.float32))
    cWy32 = np.ascontiguousarray((cbar * Wyd).astype(np.float32))
    g32 = np.ascontiguousarray(
        (t.astype(f8)[:-1][None, :NSTEP] * w_t[:, None] + beff[:, None]).astype(
            np.float32
        )
    )                                                  # [64, 1023]
    crep = np.ascontiguousarray(
        np.broadcast_to(c32[None, :], (D, NSTEP)).astype(np.float32)
    )                                                  # [64, 1023]

    in_maps = []
    for c in range(NCORES):
        xc = np.ascontiguousarray(x[c * BC : (c + 1) * BC, :].T.astype(np.float32))
        in_maps.append(
            {"xT": xc, "Wy": Wy32, "cWy": cWy32, "g": g32, "crep": crep}
        )
    return in_maps, c32


def kernel(x, t, W0, b0, W1, b1, W2, b2, W3, b3):
    from concourse.bass_utils import run_bass_kernel_spmd

    if "nc" not in _CACHE:
        _CACHE["nc"] = _build_nc()
    nc = _CACHE["nc"]

    in_maps, _ = _host_prep(x, t, W0, b0, W1, b1, W2, b2, W3, b3)
    res = run_bass_kernel_spmd(nc, in_maps, core_ids=list(range(NCORES)))
    _CACHE["last_result"] = res

    sol = np.empty((B, T, D), np.float32)
    sol[:, 0, :] = x.astype(np.float32)
    for c in range(NCORES):
        # traj [NSTEP, 64f, 64b] -> [64b, NSTEP, 64f]
        sol[c * BC : (c + 1) * BC, 1:, :] = res.results[c]["traj"].transpose(
            2, 0, 1
        )
    return sol
